# revision 9
# baseline (speedup 1.0000x reference)
"""Trainium2 Bass kernel for nn_CNNFeatMoe (CNN feature MoE with top-2 routing).

Strategy: data-parallel over batch across 8 NeuronCores (32 samples/core),
all 4 experts computed everywhere (matches the reference computation).

Precision plan:
  - conv1 (gate 64ch + 4x32 expert ch fused, M=192): split-bf16 3-term
    K-packed matmul (81 contraction rows = [x_hi|x_hi|x_lo] x [w_hi|w_lo|w_hi])
    -> ~fp32-quality output at full bf16 PE rate. The gating network needs
    near-fp32 logits so the top-2 selection matches the fp32 reference
    (a selection flip is a catastrophic absmax error).
  - gate FC + softmax/top-2 + head: exact fp32.
  - conv2 + expert FC: plain bf16 (error lands well under 1% of output scale).
Pooling: 2x2 maxpool done as tensor_tensor max on DVE straight out of PSUM
(w-parity pre-split by the conv matmuls), relu+bias applied after pooling on
the scalar engine (max/relu/per-channel-bias all commute).
"""

import numpy as np
import ml_dtypes
from contextlib import ExitStack

import concourse.bass as bass
import concourse.mybir as mybir
import concourse.tile as tile
from concourse.bass_utils import run_bass_kernel_spmd
from concourse.vector_clock import ScopedClock
from concourse.masks import make_identity
import bass_rust

F32 = mybir.dt.float32
BF16 = mybir.dt.bfloat16
AX = mybir.AxisListType
ALU = mybir.AluOpType
ACTF = mybir.ActivationFunctionType

N_CORES = 8
B = 256
S = B // N_CORES          # samples per core
C_IN, H, W = 3, 64, 64
N_EXPERTS, TOP_K, C_OUT = 4, 2, 64
HP, WP = H + 2, W + 2     # 66x66 zero-padded frame
FRAME = 34                # padded 32x32 frame for conv2 input

bf16 = ml_dtypes.bfloat16


# --------------------------------------------------------------------------
# Walrus in this environment accepts at most ONE sync wait per instruction.
# Split extra waits onto same-engine NoOps inserted right before.
# --------------------------------------------------------------------------
def _legalize_single_wait(nc):
    for _name, bbb in nc.bb_map.items():
        bb = bbb.bb if hasattr(bbb, "bb") else bbb
        insts = bb.instructions
        i = 0
        while i < len(insts):
            inst = insts[i]
            si = inst.sync_info
            if si is not None and len(si.on_wait) > 1:
                waits = list(si.on_wait)
                si.on_wait = [waits[-1]]
                for w in waits[:-1]:
                    nop = bass_rust.InstNoOp(
                        name=nc.get_next_instruction_name(), engine=inst.engine
                    )
                    nop.sync_info = mybir.SyncInfo(on_wait=[w], on_update=[])
                    nc.register_instruction(nop)
                    insts.insert(i, nop)
                    i += 1
            i += 1


class SplitWaitTileContext(tile.TileContext):
    def _drain_and_barrier(self, tick_clock, wait_clock):
        drain_inst = self.nc.sync.drain()
        wait_clock.add_sem_waits(
            drain_inst.ins, ScopedClock({None: tick_clock.global_clock})
        )
        self.nc.all_engine_barrier()
        assert self.sems is not None
        popped = self.nc._tile_sem_poison_stack.pop()
        assert popped is self._sem_poison
        self.nc.clear_and_free_semaphores(list(self.sems.allocated().values()))
        self.nc.all_engine_barrier()
        _legalize_single_wait(self.nc)


# --------------------------------------------------------------------------
# Device program
# --------------------------------------------------------------------------
def _emit(nc, tc, ctx, s_per_core):
    Sn = s_per_core
    xr_d = nc.dram_tensor("xr", [Sn, 81, HP * HP], BF16, kind="ExternalInput")
    w1_d = nc.dram_tensor("w1", [81, 192], BF16, kind="ExternalInput")
    bg_d = nc.dram_tensor("bg", [64, 1], F32, kind="ExternalInput")
    b1e_d = nc.dram_tensor("b1e", [128, 1], F32, kind="ExternalInput")
    w2_d = nc.dram_tensor("w2", [128, 9, 64], BF16, kind="ExternalInput")
    b2_d = nc.dram_tensor("b2", [64, 4], F32, kind="ExternalInput")
    efwp_d = nc.dram_tensor("efwp", [4, 128, 128, 128], BF16, kind="ExternalInput")
    efb_d = nc.dram_tensor("efb", [1, 4, 128], BF16, kind="ExternalInput")
    gfb_d = nc.dram_tensor("gfb", [1, 4], F32, kind="ExternalInput")
    gfwp_d = nc.dram_tensor("gfwp", [128, 512, 4], F32, kind="ExternalInput")
    fwt_d = nc.dram_tensor("fwt", [128, 10], F32, kind="ExternalInput")
    fb_d = nc.dram_tensor("fb", [10, 1], F32, kind="ExternalInput")
    out_d = nc.dram_tensor("out", [10, Sn], F32, kind="ExternalOutput")

    singles = ctx.enter_context(tc.tile_pool(name="singles", bufs=1))
    persist = ctx.enter_context(tc.tile_pool(name="persist", bufs=1))
    stage = ctx.enter_context(tc.tile_pool(name="stage", bufs=3))
    ps1e = ctx.enter_context(tc.tile_pool(name="ps1e", bufs=2, space="PSUM"))
    ps1g = ctx.enter_context(tc.tile_pool(name="ps1g", bufs=2, space="PSUM"))
    ps2 = ctx.enter_context(tc.tile_pool(name="ps2", bufs=1, space="PSUM"))
    psf = ctx.enter_context(tc.tile_pool(name="psf", bufs=2, space="PSUM"))
    efpool = ctx.enter_context(tc.tile_pool(name="efpool", bufs=2))

    # ---- weights / constants to SBUF ----
    w1_sb = singles.tile([81, 192], BF16)
    nc.sync.dma_start(out=w1_sb, in_=w1_d[:])
    bg_sb = singles.tile([64, 1], F32)
    nc.sync.dma_start(out=bg_sb, in_=bg_d[:])
    b1e_sb = singles.tile([128, 1], F32)
    nc.sync.dma_start(out=b1e_sb, in_=b1e_d[:])
    w2_sb = singles.tile([128, 9, 64], BF16)
    nc.sync.dma_start(out=w2_sb, in_=w2_d[:])
    b2_sb = singles.tile([64, 4], F32)
    nc.sync.dma_start(out=b2_sb, in_=b2_d[:])
    gfwp_sb = singles.tile([128, 512, 4], F32)
    nc.sync.dma_start(out=gfwp_sb, in_=gfwp_d[:])
    fwt_sb = singles.tile([128, 10], F32)
    nc.sync.dma_start(out=fwt_sb, in_=fwt_d[:])
    fb_sb = singles.tile([10, 1], F32)
    nc.sync.dma_start(out=fb_sb, in_=fb_d[:])
    efb_sb = singles.tile([1, 4, 128], BF16)
    nc.sync.dma_start(out=efb_sb, in_=efb_d[:])
    gfb_sb = singles.tile([1, 4], F32)
    nc.sync.dma_start(out=gfb_sb, in_=gfb_d[:])
    ones_bf = singles.tile([1, Sn], BF16)
    nc.vector.memset(ones_bf, 1.0)
    ones_f = singles.tile([1, Sn], F32)
    nc.vector.memset(ones_f, 1.0)
    ident = singles.tile([128, 128], F32)
    make_identity(nc, ident)

    # ---- persistent activations ----
    gpoolP = persist.tile([128, Sn, 512], F32, tag="gpoolP")
    h2P = persist.tile([128, 4, Sn, 128], BF16, tag="h2P")
    xr_sb = [persist.tile([81, HP * HP], BF16, tag=f"xr{i}", name=f"xr{i}") for i in range(2)]
    fr_sb = [persist.tile([128, FRAME * FRAME], BF16, tag=f"fr{i}", name=f"frames{i}") for i in range(2)]
    # zero frame borders once per buffer (interior is fully rewritten each use)
    for fr in fr_sb:
        f34 = fr.rearrange("p (h w) -> p h w", h=FRAME)
        nc.vector.memset(f34[:, 0, :], 0.0)
        nc.vector.memset(f34[:, 33, :], 0.0)
        nc.vector.memset(f34[:, 1:33, 0], 0.0)
        nc.vector.memset(f34[:, 1:33, 33], 0.0)

    # ---- conv phase ----
    for s in range(Sn):
        xr = xr_sb[s % 2]
        fr = fr_sb[s % 2]
        nc.sync.dma_start(out=xr, in_=xr_d[s])
        xr66 = xr.rearrange("p (h w) -> p h w", h=HP)
        f34 = fr.rearrange("p (h w) -> p h w", h=FRAME)

        for pb in range(8):          # blocks of 8 conv rows
            h0 = pb * 8
            pe = ps1e.tile([128, 2, 8, 32], F32, tag="ps1e")
            pg = ps1g.tile([64, 2, 8, 32], F32, tag="ps1g")
            for par in (0, 1):
                rhs = xr66[:, h0 : h0 + 8, par : par + 64 : 2]   # [81, 8, 32]
                nc.tensor.matmul(pe[:, par], w1_sb[:, 64:192], rhs,
                                 start=True, stop=True)
                nc.tensor.matmul(pg[:, par], w1_sb[:, 0:64], rhs,
                                 start=True, stop=True)
            # expert pooling -> frames (relu+bias after pool, cast to bf16)
            c0e = stage.tile([128, 8, 32], BF16, tag="c0e")
            nc.scalar.activation(c0e, pe[:, 0], ACTF.Copy)
            t1 = stage.tile([128, 8, 32], BF16, tag="t1")
            nc.vector.tensor_tensor(out=t1, in0=c0e, in1=pe[:, 1], op=ALU.max)
            t2 = stage.tile([128, 4, 32], BF16, tag="t2")
            nc.vector.tensor_tensor(out=t2, in0=t1[:, 0:8:2, :], in1=t1[:, 1:8:2, :],
                                    op=ALU.max)
            hp0 = h0 // 2
            nc.scalar.activation(f34[:, 1 + hp0 : 5 + hp0, 1:33], t2,
                                 ACTF.Relu, bias=b1e_sb)
            # gate pooling -> gpoolP (fp32, parity split across partition halves)
            c0g = stage.tile([64, 8, 32], F32, tag="c0g")
            nc.scalar.activation(c0g, pg[:, 0], ACTF.Copy)
            tg1 = stage.tile([64, 8, 32], F32, tag="tg1")
            nc.vector.tensor_tensor(out=tg1, in0=c0g, in1=pg[:, 1], op=ALU.max)
            tg2e = stage.tile([64, 4, 16], F32, tag="tg2e")
            tg2o = stage.tile([64, 4, 16], F32, tag="tg2o")
            nc.vector.tensor_tensor(out=tg2e, in0=tg1[:, 0:8:2, 0:32:2],
                                    in1=tg1[:, 1:8:2, 0:32:2], op=ALU.max)
            nc.vector.tensor_tensor(out=tg2o, in0=tg1[:, 0:8:2, 1:32:2],
                                    in1=tg1[:, 1:8:2, 1:32:2], op=ALU.max)
            gslice_e = gpoolP[0:64, s, hp0 * 16 : hp0 * 16 + 64]
            gslice_o = gpoolP[64:128, s, hp0 * 16 : hp0 * 16 + 64]
            nc.scalar.activation(gslice_e, tg2e, ACTF.Relu, bias=bg_sb)
            nc.scalar.activation(gslice_o, tg2o, ACTF.Relu, bias=bg_sb)

        # conv2 (kn2row, 9 accumulated shifts; experts on distinct row groups)
        for e in range(4):
            p2 = ps2.tile([64, 2, 32, 16], F32, tag="ps2")
            for par in (0, 1):
                for k9 in range(9):
                    dy, dx = k9 // 3, k9 % 3
                    rhs = f34[32 * e : 32 * e + 32, dy : dy + 32,
                              dx + par : dx + par + 31 : 2]    # [32, 32, 16]
                    nc.tensor.matmul(p2[:, par], w2_sb[32 * e : 32 * e + 32, k9, :], rhs,
                                     start=(k9 == 0), stop=(k9 == 8),
                                     tile_position=(32 * e, 0))
            c02 = stage.tile([64, 32, 16], BF16, tag="c02")
            nc.scalar.activation(c02, p2[:, 0], ACTF.Copy)
            t3 = stage.tile([64, 32, 16], BF16, tag="t3")
            nc.vector.tensor_tensor(out=t3, in0=c02, in1=p2[:, 1], op=ALU.max)
            t4e = stage.tile([64, 16, 8], BF16, tag="t4e")
            t4o = stage.tile([64, 16, 8], BF16, tag="t4o")
            nc.vector.tensor_tensor(out=t4e, in0=t3[:, 0:32:2, 0:16:2],
                                    in1=t3[:, 1:32:2, 0:16:2], op=ALU.max)
            nc.vector.tensor_tensor(out=t4o, in0=t3[:, 0:32:2, 1:16:2],
                                    in1=t3[:, 1:32:2, 1:16:2], op=ALU.max)
            nc.scalar.activation(h2P[0:64, e, s, :], t4e, ACTF.Relu,
                                 bias=b2_sb[:, e : e + 1])
            nc.scalar.activation(h2P[64:128, e, s, :], t4o, ACTF.Relu,
                                 bias=b2_sb[:, e : e + 1])

    # ---- FC phase ----
    feats = persist.tile([Sn, 4, 128], F32, tag="feats")
    for e in range(4):
        psFe = psf.tile([Sn, 128], F32, tag="acc", name=f"psF{e}")
        for jb in range(4):
            efblk = efpool.tile([128, 32, 128], BF16, tag="efblk")
            nc.sync.dma_start(out=efblk, in_=efwp_d[e, :, jb * 32 : jb * 32 + 32, :])
            for j in range(32):
                nc.tensor.matmul(psFe, h2P[:, e, :, jb * 32 + j], efblk[:, j, :],
                                 start=(jb == 0 and j == 0), stop=False)
        nc.tensor.matmul(psFe, ones_bf, efb_sb[:, e, :], start=False, stop=True)
        nc.vector.tensor_copy(out=feats[:, e], in_=psFe)
    psG = psf.tile([Sn, 4], F32, tag="acc", name="psG")
    for j in range(512):
        nc.tensor.matmul(psG, gpoolP[:, :, j], gfwp_sb[:, j, :],
                         start=(j == 0), stop=False)
    nc.tensor.matmul(psG, ones_f, gfb_sb, start=False, stop=True)

    # ---- softmax / top-2 / combine / head ----
    gs = stage.tile([Sn, 4], F32, tag="gs")
    nc.vector.tensor_copy(out=gs, in_=psG)
    mx = stage.tile([Sn, 1], F32, tag="mx")
    nc.vector.tensor_reduce(out=mx, in_=gs, axis=AX.X, op=ALU.max)
    nmx = stage.tile([Sn, 1], F32, tag="nmx")
    nc.scalar.mul(nmx, mx, -1.0)
    ex = stage.tile([Sn, 4], F32, tag="ex")
    nc.scalar.activation(ex, gs, ACTF.Exp, bias=nmx)
    cnt = stage.tile([Sn, 4], F32, tag="cnt")
    nc.vector.memset(cnt, 0.0)
    tgt = stage.tile([Sn, 4], F32, tag="tgt")
    for ep in range(4):
        col = gs[:, ep : ep + 1].broadcast_to((Sn, 4))
        nc.vector.tensor_tensor(out=tgt, in0=col, in1=gs, op=ALU.is_gt)
        nc.vector.tensor_tensor(out=cnt, in0=cnt, in1=tgt, op=ALU.add)
        if ep < 3:
            ncols = 4 - (ep + 1)
            coleq = gs[:, ep : ep + 1].broadcast_to((Sn, ncols))
            teq = stage.tile([Sn, 4], F32, tag="teq")
            nc.vector.tensor_tensor(out=teq[:, : ncols], in0=coleq,
                                    in1=gs[:, ep + 1 :], op=ALU.is_equal)
            nc.vector.tensor_tensor(out=cnt[:, ep + 1 :], in0=cnt[:, ep + 1 :],
                                    in1=teq[:, : ncols], op=ALU.add)
    mask = stage.tile([Sn, 4], F32, tag="mask")
    nc.vector.tensor_scalar(out=mask, in0=cnt, scalar1=float(TOP_K), scalar2=None,
                            op0=ALU.is_lt)
    wm = stage.tile([Sn, 4], F32, tag="wm")
    nc.vector.tensor_tensor(out=wm, in0=ex, in1=mask, op=ALU.mult)
    sw = stage.tile([Sn, 1], F32, tag="sw")
    nc.vector.tensor_reduce(out=sw, in_=wm, axis=AX.X, op=ALU.add)
    rw = stage.tile([Sn, 1], F32, tag="rw")
    nc.vector.reciprocal(rw, sw)
    wn = stage.tile([Sn, 4], F32, tag="wn")
    nc.vector.tensor_scalar(out=wn, in0=wm, scalar1=rw, scalar2=None, op0=ALU.mult)

    feat = stage.tile([Sn, 128], F32, tag="feat")
    nc.vector.tensor_scalar(out=feat, in0=feats[:, 0], scalar1=wn[:, 0:1],
                            scalar2=None, op0=ALU.mult)
    for e in range(1, 4):
        nc.vector.scalar_tensor_tensor(out=feat, in0=feats[:, e],
                                       scalar=wn[:, e : e + 1], in1=feat,
                                       op0=ALU.mult, op1=ALU.add)
    # head: out.T = fw @ feat.T   (bias per-partition on the 10 outputs)
    psT = psf.tile([128, Sn], F32, tag="acc", name="psT")
    nc.tensor.transpose(psT, feat, ident[0:Sn, 0:Sn])
    featT = stage.tile([128, Sn], F32, tag="featT")
    nc.vector.tensor_copy(out=featT, in_=psT)
    psH = psf.tile([10, Sn], F32, tag="acc", name="psH")
    nc.tensor.matmul(psH, fwt_sb, featT, start=True, stop=True)
    outT = stage.tile([10, Sn], F32, tag="outT")
    nc.scalar.activation(outT, psH, ACTF.Identity, bias=fb_sb)
    nc.sync.dma_start(out=out_d[:], in_=outT)


def build_nc(s_per_core=S):
    nc = bass.Bass()
    with ExitStack() as ctx:
        tc = ctx.enter_context(SplitWaitTileContext(nc))
        _emit(nc, tc, ctx, s_per_core)
    return nc


# --------------------------------------------------------------------------
# Host-side input preparation
# --------------------------------------------------------------------------
def _split_hi_lo(a):
    hi = a.astype(bf16)
    lo = (a - hi.astype(np.float32)).astype(bf16)
    return hi, lo


def prep_inputs(x, ew1, eb1, ew2, eb2, efw, efb, gw, gb, gfw, gfb, fw, fb):
    # x -> padded 66x66 frames, then 81-row split-bf16 im2col replicas
    xp = np.zeros((B, C_IN, HP, HP), np.float32)
    xp[:, :, 1:65, 1:65] = x
    xf = xp.reshape(B, C_IN * HP * HP)
    x_hi, x_lo = _split_hi_lo(xf)
    PADF = C_IN * HP * HP + 144
    hi_p = np.zeros((B, PADF), bf16)
    hi_p[:, : xf.shape[1]] = x_hi
    lo_p = np.zeros((B, PADF), bf16)
    lo_p[:, : xf.shape[1]] = x_lo
    xr = np.empty((B, 81, HP * HP), bf16)
    r = 0
    for t, src in ((0, hi_p), (1, hi_p), (2, lo_p)):
        for c in range(C_IN):
            for dy in range(3):
                for dx in range(3):
                    base = c * HP * HP + dy * HP + dx
                    xr[:, r] = src[:, base : base + HP * HP]
                    r += 1
    assert r == 81

    # conv1 fused weights: [81, 192] = 3 K-terms x (64 gate | 128 expert) cols
    wg = gw.transpose(1, 2, 3, 0).reshape(27, 64)            # (c,dy,dx) x o
    we = ew1.transpose(2, 3, 4, 0, 1).reshape(27, 128)       # (c,dy,dx) x (e,o)
    w_full = np.concatenate([wg, we], axis=1).astype(np.float32)
    w_hi, w_lo = _split_hi_lo(w_full)
    w1 = np.concatenate([w_hi, w_lo, w_hi], axis=0)          # [81, 192]

    bg = gb.reshape(64, 1).astype(np.float32)
    b1e = eb1.reshape(128, 1).astype(np.float32)             # (e,o) flat

    # conv2 kn2row weights: [e*32+i, k9, o=64] (expert e on partitions 32e..)
    w2 = ew2.transpose(0, 2, 3, 4, 1).reshape(128, 9, 64).astype(bf16)
    b2 = np.ascontiguousarray(eb2.transpose(1, 0).astype(np.float32))  # [64, 4]

    # expert FC weights, parity-packed to match h2P layout:
    # h2P[c + 64*par, e, s, j] with j = hp2*8 + wep2, wp2 = 2*wep2 + par
    ef = efw.reshape(4, 128, 64, 16, 8, 2)                   # e,o,c,hp2,wep2,par
    efwp = np.empty((4, 128, 128, 128), bf16)
    eft = ef.transpose(0, 5, 2, 3, 4, 1)                     # e,par,c,hp2,wep2,o
    efwp_v = efwp.reshape(4, 2, 64, 128, 128)                # e,par,c,j,o
    efwp_v[:] = eft.reshape(4, 2, 64, 128, 128)

    # gate FC weights, parity-packed to match gpoolP layout:
    # gpoolP[c + 64*par, s, j] with j = hp*16 + wep, wp = 2*wep + par
    gf = gfw.reshape(4, 64, 32, 16, 2)                       # e,c,hp,wep,par
    gfwp = np.empty((128, 512, 4), np.float32)
    gfwp_v = gfwp.reshape(2, 64, 512, 4)                     # par,c,j,e
    gfwp_v[:] = gf.transpose(4, 1, 2, 3, 0).reshape(2, 64, 512, 4)

    fwt = fw.T.astype(np.float32).copy()                     # [128, 10]
    fbv = fb.reshape(10, 1).astype(np.float32)

    shared = {
        "w1": np.ascontiguousarray(w1.astype(bf16)),
        "bg": bg, "b1e": b1e,
        "w2": np.ascontiguousarray(w2), "b2": np.ascontiguousarray(b2),
        "efwp": efwp, "gfwp": gfwp, "fwt": fwt, "fb": fbv,
        "efb": np.ascontiguousarray(efb.reshape(1, 4, 128).astype(bf16)),
        "gfb": np.ascontiguousarray(gfb.reshape(1, 4).astype(np.float32)),
    }
    return xr, shared


_NC_CACHE = {}


def kernel(**inputs):
    xr, shared = prep_inputs(**{k: np.asarray(v) for k, v in inputs.items()})
    if S not in _NC_CACHE:
        _NC_CACHE[S] = build_nc(S)
    nc = _NC_CACHE[S]
    in_maps = []
    for c in range(N_CORES):
        m = dict(shared)
        m["xr"] = np.ascontiguousarray(xr[c * S : (c + 1) * S])
        in_maps.append(m)
    res = run_bass_kernel_spmd(nc, in_maps, list(range(N_CORES)))
    out = np.empty((B, 10), np.float32)
    for c in range(N_CORES):
        out[c * S : (c + 1) * S] = res.results[c]["out"].T
    return out


# revision 11
# speedup vs baseline: 114.4819x; 114.4819x over previous
"""Trainium2 Bass kernel for nn_CNNFeatMoe (CNN feature MoE with top-2 routing).

Strategy: data-parallel over batch across 8 NeuronCores (32 samples/core),
all 4 experts computed everywhere (matches the reference computation).

Precision plan:
  - conv1 (gate 64ch + 4x32 expert ch fused, M=192): split-bf16 3-term
    K-packed matmul (81 contraction rows = [x_hi|x_hi|x_lo] x [w_hi|w_lo|w_hi])
    -> ~fp32-quality output at full bf16 PE rate. The gating network needs
    near-fp32 logits so the top-2 selection matches the fp32 reference
    (a selection flip is a catastrophic absmax error).
  - gate FC + softmax/top-2 + head: exact fp32.
  - conv2 + expert FC: plain bf16 (error lands well under 1% of output scale).
Pooling: 2x2 maxpool done as tensor_tensor max on DVE straight out of PSUM
(w-parity pre-split by the conv matmuls), relu+bias applied after pooling on
the scalar engine (max/relu/per-channel-bias all commute).
"""

import numpy as np
import ml_dtypes
from contextlib import ExitStack

import concourse.bass as bass
import concourse.mybir as mybir
import concourse.tile as tile
from concourse.vector_clock import ScopedClock
from concourse.masks import make_identity
import bass_rust

F32 = mybir.dt.float32
BF16 = mybir.dt.bfloat16
AX = mybir.AxisListType
ALU = mybir.AluOpType
ACTF = mybir.ActivationFunctionType

N_CORES = 8
B = 256
S = B // N_CORES          # samples per core
C_IN, H, W = 3, 64, 64
N_EXPERTS, TOP_K, C_OUT = 4, 2, 64
HP, WP = H + 2, W + 2     # 66x66 zero-padded frame
FRAME = 34                # padded 32x32 frame for conv2 input

bf16 = ml_dtypes.bfloat16


# --------------------------------------------------------------------------
# Walrus in this environment accepts at most ONE sync wait per instruction.
# Split extra waits onto same-engine NoOps inserted right before.
# --------------------------------------------------------------------------
def _legalize_single_wait(nc):
    for _name, bbb in nc.bb_map.items():
        bb = bbb.bb if hasattr(bbb, "bb") else bbb
        insts = bb.instructions
        i = 0
        while i < len(insts):
            inst = insts[i]
            si = inst.sync_info
            if si is not None and len(si.on_wait) > 1:
                waits = list(si.on_wait)
                si.on_wait = [waits[-1]]
                for w in waits[:-1]:
                    nop = bass_rust.InstNoOp(
                        name=nc.get_next_instruction_name(), engine=inst.engine
                    )
                    nop.sync_info = mybir.SyncInfo(on_wait=[w], on_update=[])
                    nc.register_instruction(nop)
                    insts.insert(i, nop)
                    i += 1
            i += 1


class SplitWaitTileContext(tile.TileContext):
    def _drain_and_barrier(self, tick_clock, wait_clock):
        drain_inst = self.nc.sync.drain()
        wait_clock.add_sem_waits(
            drain_inst.ins, ScopedClock({None: tick_clock.global_clock})
        )
        self.nc.all_engine_barrier()
        assert self.sems is not None
        popped = self.nc._tile_sem_poison_stack.pop()
        assert popped is self._sem_poison
        self.nc.clear_and_free_semaphores(list(self.sems.allocated().values()))
        self.nc.all_engine_barrier()
        _legalize_single_wait(self.nc)


# --------------------------------------------------------------------------
# Device program
# --------------------------------------------------------------------------
def _emit(nc, tc, ctx, s_per_core):
    Sn = s_per_core
    xr_d = nc.dram_tensor("xr", [Sn, 81, HP * HP], BF16, kind="ExternalInput")
    w1_d = nc.dram_tensor("w1", [81, 192], BF16, kind="ExternalInput")
    bg_d = nc.dram_tensor("bg", [64, 1], F32, kind="ExternalInput")
    b1e_d = nc.dram_tensor("b1e", [128, 1], F32, kind="ExternalInput")
    w2_d = nc.dram_tensor("w2", [128, 9, 64], BF16, kind="ExternalInput")
    b2_d = nc.dram_tensor("b2", [64, 4], F32, kind="ExternalInput")
    efwp_d = nc.dram_tensor("efwp", [4, 128, 128, 128], BF16, kind="ExternalInput")
    efb_d = nc.dram_tensor("efb", [1, 4, 128], BF16, kind="ExternalInput")
    gfb_d = nc.dram_tensor("gfb", [1, 4], F32, kind="ExternalInput")
    gfwp_d = nc.dram_tensor("gfwp", [128, 512, 4], F32, kind="ExternalInput")
    fwt_d = nc.dram_tensor("fwt", [128, 10], F32, kind="ExternalInput")
    fb_d = nc.dram_tensor("fb", [10, 1], F32, kind="ExternalInput")
    out_d = nc.dram_tensor("out", [10, Sn], F32, kind="ExternalOutput")

    singles = ctx.enter_context(tc.tile_pool(name="singles", bufs=1))
    persist = ctx.enter_context(tc.tile_pool(name="persist", bufs=1))
    stage = ctx.enter_context(tc.tile_pool(name="stage", bufs=3))
    ps1e = ctx.enter_context(tc.tile_pool(name="ps1e", bufs=2, space="PSUM"))
    ps1g = ctx.enter_context(tc.tile_pool(name="ps1g", bufs=2, space="PSUM"))
    ps2 = ctx.enter_context(tc.tile_pool(name="ps2", bufs=1, space="PSUM"))
    psf = ctx.enter_context(tc.tile_pool(name="psf", bufs=2, space="PSUM"))
    efpool = ctx.enter_context(tc.tile_pool(name="efpool", bufs=2))

    # ---- weights / constants to SBUF ----
    w1_sb = singles.tile([81, 192], BF16)
    nc.sync.dma_start(out=w1_sb, in_=w1_d[:])
    bg_sb = singles.tile([64, 1], F32)
    nc.sync.dma_start(out=bg_sb, in_=bg_d[:])
    b1e_sb = singles.tile([128, 1], F32)
    nc.sync.dma_start(out=b1e_sb, in_=b1e_d[:])
    w2_sb = singles.tile([128, 9, 64], BF16)
    nc.sync.dma_start(out=w2_sb, in_=w2_d[:])
    b2_sb = singles.tile([64, 4], F32)
    nc.sync.dma_start(out=b2_sb, in_=b2_d[:])
    gfwp_sb = singles.tile([128, 512, 4], F32)
    nc.sync.dma_start(out=gfwp_sb, in_=gfwp_d[:])
    fwt_sb = singles.tile([128, 10], F32)
    nc.sync.dma_start(out=fwt_sb, in_=fwt_d[:])
    fb_sb = singles.tile([10, 1], F32)
    nc.sync.dma_start(out=fb_sb, in_=fb_d[:])
    efb_sb = singles.tile([1, 4, 128], BF16)
    nc.sync.dma_start(out=efb_sb, in_=efb_d[:])
    gfb_sb = singles.tile([1, 4], F32)
    nc.sync.dma_start(out=gfb_sb, in_=gfb_d[:])
    ones_bf = singles.tile([1, Sn], BF16)
    nc.vector.memset(ones_bf, 1.0)
    ones_f = singles.tile([1, Sn], F32)
    nc.vector.memset(ones_f, 1.0)
    ident = singles.tile([128, 128], F32)
    make_identity(nc, ident)

    # ---- persistent activations ----
    gpoolP = persist.tile([128, Sn, 512], F32, tag="gpoolP")
    h2P = persist.tile([128, 4, Sn, 128], BF16, tag="h2P")
    xr_sb = [persist.tile([81, HP * HP], BF16, tag=f"xr{i}", name=f"xr{i}") for i in range(2)]
    fr_sb = [persist.tile([128, FRAME * FRAME], BF16, tag=f"fr{i}", name=f"frames{i}") for i in range(2)]
    # zero frame borders once per buffer (interior is fully rewritten each use)
    for fr in fr_sb:
        f34 = fr.rearrange("p (h w) -> p h w", h=FRAME)
        nc.vector.memset(f34[:, 0, :], 0.0)
        nc.vector.memset(f34[:, 33, :], 0.0)
        nc.vector.memset(f34[:, 1:33, 0], 0.0)
        nc.vector.memset(f34[:, 1:33, 33], 0.0)

    # ---- conv phase ----
    for s in range(Sn):
        xr = xr_sb[s % 2]
        fr = fr_sb[s % 2]
        nc.sync.dma_start(out=xr, in_=xr_d[s])
        xr66 = xr.rearrange("p (h w) -> p h w", h=HP)
        f34 = fr.rearrange("p (h w) -> p h w", h=FRAME)

        for pb in range(8):          # blocks of 8 conv rows
            h0 = pb * 8
            pe = ps1e.tile([128, 2, 8, 32], F32, tag="ps1e")
            pg = ps1g.tile([64, 2, 8, 32], F32, tag="ps1g")
            for par in (0, 1):
                rhs = xr66[:, h0 : h0 + 8, par : par + 64 : 2]   # [81, 8, 32]
                nc.tensor.matmul(pe[:, par], w1_sb[:, 64:192], rhs,
                                 start=True, stop=True)
                nc.tensor.matmul(pg[:, par], w1_sb[:, 0:64], rhs,
                                 start=True, stop=True)
            # expert pooling -> frames (relu+bias after pool, cast to bf16)
            c0e = stage.tile([128, 8, 32], BF16, tag="c0e")
            nc.scalar.activation(c0e, pe[:, 0], ACTF.Copy)
            t1 = stage.tile([128, 8, 32], BF16, tag="t1")
            nc.vector.tensor_tensor(out=t1, in0=c0e, in1=pe[:, 1], op=ALU.max)
            t2 = stage.tile([128, 4, 32], BF16, tag="t2")
            nc.vector.tensor_tensor(out=t2, in0=t1[:, 0:8:2, :], in1=t1[:, 1:8:2, :],
                                    op=ALU.max)
            hp0 = h0 // 2
            nc.scalar.activation(f34[:, 1 + hp0 : 5 + hp0, 1:33], t2,
                                 ACTF.Relu, bias=b1e_sb)
            # gate pooling -> gpoolP (fp32, parity split across partition halves)
            c0g = stage.tile([64, 8, 32], F32, tag="c0g")
            nc.scalar.activation(c0g, pg[:, 0], ACTF.Copy)
            tg1 = stage.tile([64, 8, 32], F32, tag="tg1")
            nc.vector.tensor_tensor(out=tg1, in0=c0g, in1=pg[:, 1], op=ALU.max)
            tg2e = stage.tile([64, 4, 16], F32, tag="tg2e")
            tg2o = stage.tile([64, 4, 16], F32, tag="tg2o")
            nc.vector.tensor_tensor(out=tg2e, in0=tg1[:, 0:8:2, 0:32:2],
                                    in1=tg1[:, 1:8:2, 0:32:2], op=ALU.max)
            nc.vector.tensor_tensor(out=tg2o, in0=tg1[:, 0:8:2, 1:32:2],
                                    in1=tg1[:, 1:8:2, 1:32:2], op=ALU.max)
            gslice_e = gpoolP[0:64, s, hp0 * 16 : hp0 * 16 + 64]
            gslice_o = gpoolP[64:128, s, hp0 * 16 : hp0 * 16 + 64]
            nc.scalar.activation(gslice_e, tg2e, ACTF.Relu, bias=bg_sb)
            nc.scalar.activation(gslice_o, tg2o, ACTF.Relu, bias=bg_sb)

        # conv2 (kn2row, 9 accumulated shifts; experts on distinct row groups)
        for e in range(4):
            p2 = ps2.tile([64, 2, 32, 16], F32, tag="ps2")
            for par in (0, 1):
                for k9 in range(9):
                    dy, dx = k9 // 3, k9 % 3
                    rhs = f34[32 * e : 32 * e + 32, dy : dy + 32,
                              dx + par : dx + par + 31 : 2]    # [32, 32, 16]
                    nc.tensor.matmul(p2[:, par], w2_sb[32 * e : 32 * e + 32, k9, :], rhs,
                                     start=(k9 == 0), stop=(k9 == 8),
                                     tile_position=(32 * e, 0))
            c02 = stage.tile([64, 32, 16], BF16, tag="c02")
            nc.scalar.activation(c02, p2[:, 0], ACTF.Copy)
            t3 = stage.tile([64, 32, 16], BF16, tag="t3")
            nc.vector.tensor_tensor(out=t3, in0=c02, in1=p2[:, 1], op=ALU.max)
            t4e = stage.tile([64, 16, 8], BF16, tag="t4e")
            t4o = stage.tile([64, 16, 8], BF16, tag="t4o")
            nc.vector.tensor_tensor(out=t4e, in0=t3[:, 0:32:2, 0:16:2],
                                    in1=t3[:, 1:32:2, 0:16:2], op=ALU.max)
            nc.vector.tensor_tensor(out=t4o, in0=t3[:, 0:32:2, 1:16:2],
                                    in1=t3[:, 1:32:2, 1:16:2], op=ALU.max)
            nc.scalar.activation(h2P[0:64, e, s, :], t4e, ACTF.Relu,
                                 bias=b2_sb[:, e : e + 1])
            nc.scalar.activation(h2P[64:128, e, s, :], t4o, ACTF.Relu,
                                 bias=b2_sb[:, e : e + 1])

    # ---- FC phase ----
    feats = persist.tile([Sn, 4, 128], F32, tag="feats")
    for e in range(4):
        psFe = psf.tile([Sn, 128], F32, tag="acc", name=f"psF{e}")
        for jb in range(4):
            efblk = efpool.tile([128, 32, 128], BF16, tag="efblk")
            nc.sync.dma_start(out=efblk, in_=efwp_d[e, :, jb * 32 : jb * 32 + 32, :])
            for j in range(32):
                nc.tensor.matmul(psFe, h2P[:, e, :, jb * 32 + j], efblk[:, j, :],
                                 start=(jb == 0 and j == 0), stop=False)
        nc.tensor.matmul(psFe, ones_bf, efb_sb[:, e, :], start=False, stop=True)
        nc.vector.tensor_copy(out=feats[:, e], in_=psFe)
    psG = psf.tile([Sn, 4], F32, tag="acc", name="psG")
    for j in range(512):
        nc.tensor.matmul(psG, gpoolP[:, :, j], gfwp_sb[:, j, :],
                         start=(j == 0), stop=False)
    nc.tensor.matmul(psG, ones_f, gfb_sb, start=False, stop=True)

    # ---- softmax / top-2 / combine / head ----
    gs = stage.tile([Sn, 4], F32, tag="gs")
    nc.vector.tensor_copy(out=gs, in_=psG)
    mx = stage.tile([Sn, 1], F32, tag="mx")
    nc.vector.tensor_reduce(out=mx, in_=gs, axis=AX.X, op=ALU.max)
    nmx = stage.tile([Sn, 1], F32, tag="nmx")
    nc.scalar.mul(nmx, mx, -1.0)
    ex = stage.tile([Sn, 4], F32, tag="ex")
    nc.scalar.activation(ex, gs, ACTF.Exp, bias=nmx)
    cnt = stage.tile([Sn, 4], F32, tag="cnt")
    nc.vector.memset(cnt, 0.0)
    tgt = stage.tile([Sn, 4], F32, tag="tgt")
    for ep in range(4):
        col = gs[:, ep : ep + 1].broadcast_to((Sn, 4))
        nc.vector.tensor_tensor(out=tgt, in0=col, in1=gs, op=ALU.is_gt)
        nc.vector.tensor_tensor(out=cnt, in0=cnt, in1=tgt, op=ALU.add)
        if ep < 3:
            ncols = 4 - (ep + 1)
            coleq = gs[:, ep : ep + 1].broadcast_to((Sn, ncols))
            teq = stage.tile([Sn, 4], F32, tag="teq")
            nc.vector.tensor_tensor(out=teq[:, : ncols], in0=coleq,
                                    in1=gs[:, ep + 1 :], op=ALU.is_equal)
            nc.vector.tensor_tensor(out=cnt[:, ep + 1 :], in0=cnt[:, ep + 1 :],
                                    in1=teq[:, : ncols], op=ALU.add)
    mask = stage.tile([Sn, 4], F32, tag="mask")
    nc.vector.tensor_scalar(out=mask, in0=cnt, scalar1=float(TOP_K), scalar2=None,
                            op0=ALU.is_lt)
    wm = stage.tile([Sn, 4], F32, tag="wm")
    nc.vector.tensor_tensor(out=wm, in0=ex, in1=mask, op=ALU.mult)
    sw = stage.tile([Sn, 1], F32, tag="sw")
    nc.vector.tensor_reduce(out=sw, in_=wm, axis=AX.X, op=ALU.add)
    rw = stage.tile([Sn, 1], F32, tag="rw")
    nc.vector.reciprocal(rw, sw)
    wn = stage.tile([Sn, 4], F32, tag="wn")
    nc.vector.tensor_scalar(out=wn, in0=wm, scalar1=rw, scalar2=None, op0=ALU.mult)

    feat = stage.tile([Sn, 128], F32, tag="feat")
    nc.vector.tensor_scalar(out=feat, in0=feats[:, 0], scalar1=wn[:, 0:1],
                            scalar2=None, op0=ALU.mult)
    for e in range(1, 4):
        nc.vector.scalar_tensor_tensor(out=feat, in0=feats[:, e],
                                       scalar=wn[:, e : e + 1], in1=feat,
                                       op0=ALU.mult, op1=ALU.add)
    # head: out.T = fw @ feat.T   (bias per-partition on the 10 outputs)
    psT = psf.tile([128, Sn], F32, tag="acc", name="psT")
    nc.tensor.transpose(psT, feat, ident[0:Sn, 0:Sn])
    featT = stage.tile([128, Sn], F32, tag="featT")
    nc.vector.tensor_copy(out=featT, in_=psT)
    psH = psf.tile([10, Sn], F32, tag="acc", name="psH")
    nc.tensor.matmul(psH, fwt_sb, featT, start=True, stop=True)
    outT = stage.tile([10, Sn], F32, tag="outT")
    nc.scalar.activation(outT, psH, ACTF.Identity, bias=fb_sb)
    nc.sync.dma_start(out=out_d[:], in_=outT)


def build_nc(s_per_core=S):
    nc = bass.Bass()
    with ExitStack() as ctx:
        tc = ctx.enter_context(SplitWaitTileContext(nc))
        _emit(nc, tc, ctx, s_per_core)
    return nc


# --------------------------------------------------------------------------
# Host-side input preparation
# --------------------------------------------------------------------------
def _split_hi_lo(a):
    hi = a.astype(bf16)
    lo = (a - hi.astype(np.float32)).astype(bf16)
    return hi, lo


def prep_inputs(x, ew1, eb1, ew2, eb2, efw, efb, gw, gb, gfw, gfb, fw, fb):
    # x -> padded 66x66 frames, then 81-row split-bf16 im2col replicas
    xp = np.zeros((B, C_IN, HP, HP), np.float32)
    xp[:, :, 1:65, 1:65] = x
    xf = xp.reshape(B, C_IN * HP * HP)
    x_hi, x_lo = _split_hi_lo(xf)
    PADF = C_IN * HP * HP + 144
    hi_p = np.zeros((B, PADF), bf16)
    hi_p[:, : xf.shape[1]] = x_hi
    lo_p = np.zeros((B, PADF), bf16)
    lo_p[:, : xf.shape[1]] = x_lo
    xr = np.empty((B, 81, HP * HP), bf16)
    r = 0
    for t, src in ((0, hi_p), (1, hi_p), (2, lo_p)):
        for c in range(C_IN):
            for dy in range(3):
                for dx in range(3):
                    base = c * HP * HP + dy * HP + dx
                    xr[:, r] = src[:, base : base + HP * HP]
                    r += 1
    assert r == 81

    # conv1 fused weights: [81, 192] = 3 K-terms x (64 gate | 128 expert) cols
    wg = gw.transpose(1, 2, 3, 0).reshape(27, 64)            # (c,dy,dx) x o
    we = ew1.transpose(2, 3, 4, 0, 1).reshape(27, 128)       # (c,dy,dx) x (e,o)
    w_full = np.concatenate([wg, we], axis=1).astype(np.float32)
    w_hi, w_lo = _split_hi_lo(w_full)
    w1 = np.concatenate([w_hi, w_lo, w_hi], axis=0)          # [81, 192]

    bg = gb.reshape(64, 1).astype(np.float32)
    b1e = eb1.reshape(128, 1).astype(np.float32)             # (e,o) flat

    # conv2 kn2row weights: [e*32+i, k9, o=64] (expert e on partitions 32e..)
    w2 = ew2.transpose(0, 2, 3, 4, 1).reshape(128, 9, 64).astype(bf16)
    b2 = np.ascontiguousarray(eb2.transpose(1, 0).astype(np.float32))  # [64, 4]

    # expert FC weights, parity-packed to match h2P layout:
    # h2P[c + 64*par, e, s, j] with j = hp2*8 + wep2, wp2 = 2*wep2 + par
    ef = efw.reshape(4, 128, 64, 16, 8, 2)                   # e,o,c,hp2,wep2,par
    efwp = np.empty((4, 128, 128, 128), bf16)
    eft = ef.transpose(0, 5, 2, 3, 4, 1)                     # e,par,c,hp2,wep2,o
    efwp_v = efwp.reshape(4, 2, 64, 128, 128)                # e,par,c,j,o
    efwp_v[:] = eft.reshape(4, 2, 64, 128, 128)

    # gate FC weights, parity-packed to match gpoolP layout:
    # gpoolP[c + 64*par, s, j] with j = hp*16 + wep, wp = 2*wep + par
    gf = gfw.reshape(4, 64, 32, 16, 2)                       # e,c,hp,wep,par
    gfwp = np.empty((128, 512, 4), np.float32)
    gfwp_v = gfwp.reshape(2, 64, 512, 4)                     # par,c,j,e
    gfwp_v[:] = gf.transpose(4, 1, 2, 3, 0).reshape(2, 64, 512, 4)

    fwt = fw.T.astype(np.float32).copy()                     # [128, 10]
    fbv = fb.reshape(10, 1).astype(np.float32)

    shared = {
        "w1": np.ascontiguousarray(w1.astype(bf16)),
        "bg": bg, "b1e": b1e,
        "w2": np.ascontiguousarray(w2), "b2": np.ascontiguousarray(b2),
        "efwp": efwp, "gfwp": gfwp, "fwt": fwt, "fb": fbv,
        "efb": np.ascontiguousarray(efb.reshape(1, 4, 128).astype(bf16)),
        "gfb": np.ascontiguousarray(gfb.reshape(1, 4).astype(np.float32)),
    }
    return xr, shared




# --------------------------------------------------------------------------
# Persistent runner: trace/compile the NEFF-wrapped jax function once, then
# reuse it (repeat kernel() calls skip XLA retracing; test.py can time with
# device-resident inputs).
# --------------------------------------------------------------------------
class _Runner:
    def __init__(self, nc, n_cores):
        import jax
        from jax.experimental.shard_map import shard_map
        from jax.sharding import Mesh, PartitionSpec, NamedSharding
        from concourse import bass2jax

        bass2jax.install_neuronx_cc_hook()
        self.jax = jax
        self.nc = nc
        self.n_cores = n_cores
        partition_name = (
            nc.partition_id_tensor.name if nc.partition_id_tensor else None
        )
        in_names, out_names, out_avals, zero_outs = [], [], [], []
        for alloc in nc.m.functions[0].allocations:
            if not isinstance(alloc, mybir.MemoryLocationSet):
                continue
            name = alloc.memorylocations[0].name
            if alloc.kind == "ExternalInput":
                if name == partition_name:
                    continue
                in_names.append(name)
            elif alloc.kind == "ExternalOutput":
                out_names.append(name)
                shape = tuple(alloc.tensor_shape)
                dtype = mybir.dt.np(alloc.dtype)
                out_avals.append(jax.core.ShapedArray(shape, dtype))
                zero_outs.append(np.zeros(shape, dtype))
        self.in_names = list(in_names)
        self.out_names = out_names
        self.zero_outs = zero_outs
        n_params = len(in_names)
        all_names = in_names + out_names
        if partition_name is not None:
            all_names = all_names + [partition_name]
        donate = tuple(range(n_params, n_params + len(out_names)))
        out_avals_t = tuple(out_avals)

        def _body(*args):
            operands = list(args)
            if partition_name is not None:
                operands.append(bass2jax.partition_id_tensor())
            outs = bass2jax._bass_exec_p.bind(
                *operands,
                out_avals=out_avals_t,
                in_names=tuple(all_names),
                out_names=tuple(out_names),
                lowering_input_output_aliases=(),
                sim_require_finite=True,
                sim_require_nnan=True,
                nc=nc,
            )
            return tuple(outs)

        devices = jax.devices()[:n_cores]
        self.mesh = Mesh(np.asarray(devices), ("core",))
        self.sharding = NamedSharding(self.mesh, PartitionSpec("core"))
        in_specs = (PartitionSpec("core"),) * (n_params + len(out_names))
        out_specs = (PartitionSpec("core"),) * len(out_names)
        self.fn = jax.jit(
            shard_map(_body, mesh=self.mesh, in_specs=in_specs,
                      out_specs=out_specs, check_rep=False),
            donate_argnums=donate, keep_unused=True,
        )

    def concat_inputs(self, in_maps):
        return [
            np.concatenate([np.asarray(m[name]) for m in in_maps], axis=0)
            for name in self.in_names
        ]

    def put(self, concat_in):
        return [self.jax.device_put(a, self.sharding) for a in concat_in]

    def call(self, device_in):
        zeros = [np.zeros((self.n_cores * z.shape[0], *z.shape[1:]), z.dtype)
                 for z in self.zero_outs]
        outs = self.fn(*device_in, *zeros)
        return outs

    def run(self, in_maps):
        outs = self.call(self.put(self.concat_inputs(in_maps)))
        n = self.n_cores
        res = []
        for c in range(n):
            d = {}
            for i, name in enumerate(self.out_names):
                full = np.asarray(outs[i])
                d[name] = full.reshape(n, full.shape[0] // n, *full.shape[1:])[c]
            res.append(d)
        return res


_RUNNER = None


def get_runner():
    global _RUNNER
    if _RUNNER is None:
        _RUNNER = _Runner(build_nc(S), N_CORES)
    return _RUNNER


def kernel(**inputs):
    xr, shared = prep_inputs(**{k: np.asarray(v) for k, v in inputs.items()})
    runner = get_runner()
    in_maps = []
    for c in range(N_CORES):
        m = dict(shared)
        m["xr"] = np.ascontiguousarray(xr[c * S : (c + 1) * S])
        in_maps.append(m)
    res = runner.run(in_maps)
    out = np.empty((B, 10), np.float32)
    for c in range(N_CORES):
        out[c * S : (c + 1) * S] = res[c]["out"].T
    return out


# revision 12
# speedup vs baseline: 4578.5760x; 39.9939x over previous
"""Trainium2 Bass kernel for nn_CNNFeatMoe (CNN feature MoE with top-2 routing).

Strategy: data-parallel over batch across 8 NeuronCores (32 samples/core),
all 4 experts computed everywhere (matches the reference computation).

Precision plan:
  - conv1 (gate 64ch + 4x32 expert ch fused, M=192): split-bf16 3-term
    K-packed matmul (81 contraction rows = [x_hi|x_hi|x_lo] x [w_hi|w_lo|w_hi])
    -> ~fp32-quality output at full bf16 PE rate. The gating network needs
    near-fp32 logits so the top-2 selection matches the fp32 reference
    (a selection flip is a catastrophic absmax error).
  - gate FC + softmax/top-2 + head: exact fp32.
  - conv2 + expert FC: plain bf16 (error lands well under 1% of output scale).
Pooling: 2x2 maxpool done as tensor_tensor max on DVE straight out of PSUM
(w-parity pre-split by the conv matmuls), relu+bias applied after pooling on
the scalar engine (max/relu/per-channel-bias all commute).
"""

import numpy as np
import ml_dtypes
from contextlib import ExitStack

import concourse.bass as bass
import concourse.mybir as mybir
import concourse.tile as tile
from concourse.vector_clock import ScopedClock
from concourse.masks import make_identity
import bass_rust

F32 = mybir.dt.float32
BF16 = mybir.dt.bfloat16
AX = mybir.AxisListType
ALU = mybir.AluOpType
ACTF = mybir.ActivationFunctionType

N_CORES = 8
B = 256
S = B // N_CORES          # samples per core
C_IN, H, W = 3, 64, 64
N_EXPERTS, TOP_K, C_OUT = 4, 2, 64
HP, WP = H + 2, W + 2     # 66x66 zero-padded frame
FRAME = 34                # padded 32x32 frame for conv2 input

bf16 = ml_dtypes.bfloat16


# --------------------------------------------------------------------------
# Walrus in this environment accepts at most ONE sync wait per instruction.
# Split extra waits onto same-engine NoOps inserted right before.
# --------------------------------------------------------------------------
def _legalize_single_wait(nc):
    for _name, bbb in nc.bb_map.items():
        bb = bbb.bb if hasattr(bbb, "bb") else bbb
        insts = bb.instructions
        i = 0
        while i < len(insts):
            inst = insts[i]
            si = inst.sync_info
            if si is not None and len(si.on_wait) > 1:
                waits = list(si.on_wait)
                si.on_wait = [waits[-1]]
                for w in waits[:-1]:
                    nop = bass_rust.InstNoOp(
                        name=nc.get_next_instruction_name(), engine=inst.engine
                    )
                    nop.sync_info = mybir.SyncInfo(on_wait=[w], on_update=[])
                    nc.register_instruction(nop)
                    insts.insert(i, nop)
                    i += 1
            i += 1


class SplitWaitTileContext(tile.TileContext):
    def _drain_and_barrier(self, tick_clock, wait_clock):
        drain_inst = self.nc.sync.drain()
        wait_clock.add_sem_waits(
            drain_inst.ins, ScopedClock({None: tick_clock.global_clock})
        )
        self.nc.all_engine_barrier()
        assert self.sems is not None
        popped = self.nc._tile_sem_poison_stack.pop()
        assert popped is self._sem_poison
        self.nc.clear_and_free_semaphores(list(self.sems.allocated().values()))
        self.nc.all_engine_barrier()
        _legalize_single_wait(self.nc)


# --------------------------------------------------------------------------
# Device program
# --------------------------------------------------------------------------
def _emit(nc, tc, ctx, s_per_core, loop_n=1):
    Sn = s_per_core
    xr_d = nc.dram_tensor("xr", [Sn, 81, HP * HP], BF16, kind="ExternalInput")
    w1_d = nc.dram_tensor("w1", [81, 192], BF16, kind="ExternalInput")
    bg_d = nc.dram_tensor("bg", [64, 1], F32, kind="ExternalInput")
    b1e_d = nc.dram_tensor("b1e", [128, 1], F32, kind="ExternalInput")
    w2_d = nc.dram_tensor("w2", [128, 9, 64], BF16, kind="ExternalInput")
    b2_d = nc.dram_tensor("b2", [64, 4], F32, kind="ExternalInput")
    efwp_d = nc.dram_tensor("efwp", [4, 128, 128, 128], BF16, kind="ExternalInput")
    efb_d = nc.dram_tensor("efb", [1, 4, 128], BF16, kind="ExternalInput")
    gfb_d = nc.dram_tensor("gfb", [1, 4], F32, kind="ExternalInput")
    gfwp_d = nc.dram_tensor("gfwp", [128, 512, 4], F32, kind="ExternalInput")
    fwt_d = nc.dram_tensor("fwt", [128, 10], F32, kind="ExternalInput")
    fb_d = nc.dram_tensor("fb", [10, 1], F32, kind="ExternalInput")
    out_d = nc.dram_tensor("out", [10, Sn], F32, kind="ExternalOutput")

    singles = ctx.enter_context(tc.tile_pool(name="singles", bufs=1))
    persist = ctx.enter_context(tc.tile_pool(name="persist", bufs=1))
    stage = ctx.enter_context(tc.tile_pool(name="stage", bufs=3))
    ps1e = ctx.enter_context(tc.tile_pool(name="ps1e", bufs=2, space="PSUM"))
    ps1g = ctx.enter_context(tc.tile_pool(name="ps1g", bufs=2, space="PSUM"))
    ps2 = ctx.enter_context(tc.tile_pool(name="ps2", bufs=1, space="PSUM"))
    psf = ctx.enter_context(tc.tile_pool(name="psf", bufs=2, space="PSUM"))
    efpool = ctx.enter_context(tc.tile_pool(name="efpool", bufs=2))

    # ---- weights / constants to SBUF ----
    w1_sb = singles.tile([81, 192], BF16)
    nc.sync.dma_start(out=w1_sb, in_=w1_d[:])
    bg_sb = singles.tile([64, 1], F32)
    nc.sync.dma_start(out=bg_sb, in_=bg_d[:])
    b1e_sb = singles.tile([128, 1], F32)
    nc.sync.dma_start(out=b1e_sb, in_=b1e_d[:])
    w2_sb = singles.tile([128, 9, 64], BF16)
    nc.sync.dma_start(out=w2_sb, in_=w2_d[:])
    b2_sb = singles.tile([64, 4], F32)
    nc.sync.dma_start(out=b2_sb, in_=b2_d[:])
    gfwp_sb = singles.tile([128, 512, 4], F32)
    nc.sync.dma_start(out=gfwp_sb, in_=gfwp_d[:])
    fwt_sb = singles.tile([128, 10], F32)
    nc.sync.dma_start(out=fwt_sb, in_=fwt_d[:])
    fb_sb = singles.tile([10, 1], F32)
    nc.sync.dma_start(out=fb_sb, in_=fb_d[:])
    efb_sb = singles.tile([1, 4, 128], BF16)
    nc.sync.dma_start(out=efb_sb, in_=efb_d[:])
    gfb_sb = singles.tile([1, 4], F32)
    nc.sync.dma_start(out=gfb_sb, in_=gfb_d[:])
    ones_bf = singles.tile([1, Sn], BF16)
    nc.vector.memset(ones_bf, 1.0)
    ones_f = singles.tile([1, Sn], F32)
    nc.vector.memset(ones_f, 1.0)
    ident = singles.tile([128, 128], F32)
    make_identity(nc, ident)

    # ---- persistent activations ----
    gpoolP = persist.tile([128, Sn, 512], F32, tag="gpoolP")
    h2P = persist.tile([128, 4, Sn, 128], BF16, tag="h2P")
    xr_sb = [persist.tile([81, HP * HP], BF16, tag=f"xr{i}", name=f"xr{i}") for i in range(2)]
    fr_sb = [persist.tile([128, FRAME * FRAME], BF16, tag=f"fr{i}", name=f"frames{i}") for i in range(2)]
    # zero frame borders once per buffer (interior is fully rewritten each use)
    for fr in fr_sb:
        f34 = fr.rearrange("p (h w) -> p h w", h=FRAME)
        nc.vector.memset(f34[:, 0, :], 0.0)
        nc.vector.memset(f34[:, 33, :], 0.0)
        nc.vector.memset(f34[:, 1:33, 0], 0.0)
        nc.vector.memset(f34[:, 1:33, 33], 0.0)

    # ---- conv phase (optionally repeated on-device for timing) ----
    loop_cm = tc.For_i(0, loop_n, 1) if loop_n > 1 else None
    if loop_cm is not None:
        ctx.enter_context(loop_cm)
    for s in range(Sn):
        xr = xr_sb[s % 2]
        fr = fr_sb[s % 2]
        nc.sync.dma_start(out=xr, in_=xr_d[s])
        xr66 = xr.rearrange("p (h w) -> p h w", h=HP)
        f34 = fr.rearrange("p (h w) -> p h w", h=FRAME)

        for pb in range(8):          # blocks of 8 conv rows
            h0 = pb * 8
            pe = ps1e.tile([128, 2, 8, 32], F32, tag="ps1e")
            pg = ps1g.tile([64, 2, 8, 32], F32, tag="ps1g")
            for par in (0, 1):
                rhs = xr66[:, h0 : h0 + 8, par : par + 64 : 2]   # [81, 8, 32]
                nc.tensor.matmul(pe[:, par], w1_sb[:, 64:192], rhs,
                                 start=True, stop=True)
                nc.tensor.matmul(pg[:, par], w1_sb[:, 0:64], rhs,
                                 start=True, stop=True)
            # expert pooling -> frames (relu+bias after pool, cast to bf16)
            c0e = stage.tile([128, 8, 32], BF16, tag="c0e")
            nc.scalar.activation(c0e, pe[:, 0], ACTF.Copy)
            t1 = stage.tile([128, 8, 32], BF16, tag="t1")
            nc.vector.tensor_tensor(out=t1, in0=c0e, in1=pe[:, 1], op=ALU.max)
            t2 = stage.tile([128, 4, 32], BF16, tag="t2")
            nc.vector.tensor_tensor(out=t2, in0=t1[:, 0:8:2, :], in1=t1[:, 1:8:2, :],
                                    op=ALU.max)
            hp0 = h0 // 2
            nc.scalar.activation(f34[:, 1 + hp0 : 5 + hp0, 1:33], t2,
                                 ACTF.Relu, bias=b1e_sb)
            # gate pooling -> gpoolP (fp32, parity split across partition halves)
            c0g = stage.tile([64, 8, 32], F32, tag="c0g")
            nc.scalar.activation(c0g, pg[:, 0], ACTF.Copy)
            tg1 = stage.tile([64, 8, 32], F32, tag="tg1")
            nc.vector.tensor_tensor(out=tg1, in0=c0g, in1=pg[:, 1], op=ALU.max)
            tg2e = stage.tile([64, 4, 16], F32, tag="tg2e")
            tg2o = stage.tile([64, 4, 16], F32, tag="tg2o")
            nc.vector.tensor_tensor(out=tg2e, in0=tg1[:, 0:8:2, 0:32:2],
                                    in1=tg1[:, 1:8:2, 0:32:2], op=ALU.max)
            nc.vector.tensor_tensor(out=tg2o, in0=tg1[:, 0:8:2, 1:32:2],
                                    in1=tg1[:, 1:8:2, 1:32:2], op=ALU.max)
            gslice_e = gpoolP[0:64, s, hp0 * 16 : hp0 * 16 + 64]
            gslice_o = gpoolP[64:128, s, hp0 * 16 : hp0 * 16 + 64]
            nc.scalar.activation(gslice_e, tg2e, ACTF.Relu, bias=bg_sb)
            nc.scalar.activation(gslice_o, tg2o, ACTF.Relu, bias=bg_sb)

        # conv2 (kn2row, 9 accumulated shifts; experts on distinct row groups)
        for e in range(4):
            p2 = ps2.tile([64, 2, 32, 16], F32, tag="ps2")
            for par in (0, 1):
                for k9 in range(9):
                    dy, dx = k9 // 3, k9 % 3
                    rhs = f34[32 * e : 32 * e + 32, dy : dy + 32,
                              dx + par : dx + par + 31 : 2]    # [32, 32, 16]
                    nc.tensor.matmul(p2[:, par], w2_sb[32 * e : 32 * e + 32, k9, :], rhs,
                                     start=(k9 == 0), stop=(k9 == 8),
                                     tile_position=(32 * e, 0))
            c02 = stage.tile([64, 32, 16], BF16, tag="c02")
            nc.scalar.activation(c02, p2[:, 0], ACTF.Copy)
            t3 = stage.tile([64, 32, 16], BF16, tag="t3")
            nc.vector.tensor_tensor(out=t3, in0=c02, in1=p2[:, 1], op=ALU.max)
            t4e = stage.tile([64, 16, 8], BF16, tag="t4e")
            t4o = stage.tile([64, 16, 8], BF16, tag="t4o")
            nc.vector.tensor_tensor(out=t4e, in0=t3[:, 0:32:2, 0:16:2],
                                    in1=t3[:, 1:32:2, 0:16:2], op=ALU.max)
            nc.vector.tensor_tensor(out=t4o, in0=t3[:, 0:32:2, 1:16:2],
                                    in1=t3[:, 1:32:2, 1:16:2], op=ALU.max)
            nc.scalar.activation(h2P[0:64, e, s, :], t4e, ACTF.Relu,
                                 bias=b2_sb[:, e : e + 1])
            nc.scalar.activation(h2P[64:128, e, s, :], t4o, ACTF.Relu,
                                 bias=b2_sb[:, e : e + 1])

    # ---- FC phase ----
    feats = persist.tile([Sn, 4, 128], F32, tag="feats")
    for e in range(4):
        psFe = psf.tile([Sn, 128], F32, tag="acc", name=f"psF{e}")
        for jb in range(4):
            efblk = efpool.tile([128, 32, 128], BF16, tag="efblk")
            nc.sync.dma_start(out=efblk, in_=efwp_d[e, :, jb * 32 : jb * 32 + 32, :])
            for j in range(32):
                nc.tensor.matmul(psFe, h2P[:, e, :, jb * 32 + j], efblk[:, j, :],
                                 start=(jb == 0 and j == 0), stop=False)
        nc.tensor.matmul(psFe, ones_bf, efb_sb[:, e, :], start=False, stop=True)
        nc.vector.tensor_copy(out=feats[:, e], in_=psFe)
    psG = psf.tile([Sn, 4], F32, tag="acc", name="psG")
    for j in range(512):
        nc.tensor.matmul(psG, gpoolP[:, :, j], gfwp_sb[:, j, :],
                         start=(j == 0), stop=False)
    nc.tensor.matmul(psG, ones_f, gfb_sb, start=False, stop=True)

    # ---- softmax / top-2 / combine / head ----
    gs = stage.tile([Sn, 4], F32, tag="gs")
    nc.vector.tensor_copy(out=gs, in_=psG)
    mx = stage.tile([Sn, 1], F32, tag="mx")
    nc.vector.tensor_reduce(out=mx, in_=gs, axis=AX.X, op=ALU.max)
    nmx = stage.tile([Sn, 1], F32, tag="nmx")
    nc.scalar.mul(nmx, mx, -1.0)
    ex = stage.tile([Sn, 4], F32, tag="ex")
    nc.scalar.activation(ex, gs, ACTF.Exp, bias=nmx)
    cnt = stage.tile([Sn, 4], F32, tag="cnt")
    nc.vector.memset(cnt, 0.0)
    tgt = stage.tile([Sn, 4], F32, tag="tgt")
    for ep in range(4):
        col = gs[:, ep : ep + 1].broadcast_to((Sn, 4))
        nc.vector.tensor_tensor(out=tgt, in0=col, in1=gs, op=ALU.is_gt)
        nc.vector.tensor_tensor(out=cnt, in0=cnt, in1=tgt, op=ALU.add)
        if ep < 3:
            ncols = 4 - (ep + 1)
            coleq = gs[:, ep : ep + 1].broadcast_to((Sn, ncols))
            teq = stage.tile([Sn, 4], F32, tag="teq")
            nc.vector.tensor_tensor(out=teq[:, : ncols], in0=coleq,
                                    in1=gs[:, ep + 1 :], op=ALU.is_equal)
            nc.vector.tensor_tensor(out=cnt[:, ep + 1 :], in0=cnt[:, ep + 1 :],
                                    in1=teq[:, : ncols], op=ALU.add)
    mask = stage.tile([Sn, 4], F32, tag="mask")
    nc.vector.tensor_scalar(out=mask, in0=cnt, scalar1=float(TOP_K), scalar2=None,
                            op0=ALU.is_lt)
    wm = stage.tile([Sn, 4], F32, tag="wm")
    nc.vector.tensor_tensor(out=wm, in0=ex, in1=mask, op=ALU.mult)
    sw = stage.tile([Sn, 1], F32, tag="sw")
    nc.vector.tensor_reduce(out=sw, in_=wm, axis=AX.X, op=ALU.add)
    rw = stage.tile([Sn, 1], F32, tag="rw")
    nc.vector.reciprocal(rw, sw)
    wn = stage.tile([Sn, 4], F32, tag="wn")
    nc.vector.tensor_scalar(out=wn, in0=wm, scalar1=rw, scalar2=None, op0=ALU.mult)

    feat = stage.tile([Sn, 128], F32, tag="feat")
    nc.vector.tensor_scalar(out=feat, in0=feats[:, 0], scalar1=wn[:, 0:1],
                            scalar2=None, op0=ALU.mult)
    for e in range(1, 4):
        nc.vector.scalar_tensor_tensor(out=feat, in0=feats[:, e],
                                       scalar=wn[:, e : e + 1], in1=feat,
                                       op0=ALU.mult, op1=ALU.add)
    # head: out.T = fw @ feat.T   (bias per-partition on the 10 outputs)
    psT = psf.tile([128, Sn], F32, tag="acc", name="psT")
    nc.tensor.transpose(psT, feat, ident[0:Sn, 0:Sn])
    featT = stage.tile([128, Sn], F32, tag="featT")
    nc.vector.tensor_copy(out=featT, in_=psT)
    psH = psf.tile([10, Sn], F32, tag="acc", name="psH")
    nc.tensor.matmul(psH, fwt_sb, featT, start=True, stop=True)
    outT = stage.tile([10, Sn], F32, tag="outT")
    nc.scalar.activation(outT, psH, ACTF.Identity, bias=fb_sb)
    nc.sync.dma_start(out=out_d[:], in_=outT)


def build_nc(s_per_core=S, loop_n=1):
    nc = bass.Bass()
    with ExitStack() as ctx:
        tc = ctx.enter_context(SplitWaitTileContext(nc))
        _emit(nc, tc, ctx, s_per_core, loop_n=loop_n)
    return nc


# --------------------------------------------------------------------------
# Host-side input preparation
# --------------------------------------------------------------------------
def _split_hi_lo(a):
    hi = a.astype(bf16)
    lo = (a - hi.astype(np.float32)).astype(bf16)
    return hi, lo


def prep_inputs(x, ew1, eb1, ew2, eb2, efw, efb, gw, gb, gfw, gfb, fw, fb):
    # x -> padded 66x66 frames, then 81-row split-bf16 im2col replicas
    xp = np.zeros((B, C_IN, HP, HP), np.float32)
    xp[:, :, 1:65, 1:65] = x
    xf = xp.reshape(B, C_IN * HP * HP)
    x_hi, x_lo = _split_hi_lo(xf)
    PADF = C_IN * HP * HP + 144
    hi_p = np.zeros((B, PADF), bf16)
    hi_p[:, : xf.shape[1]] = x_hi
    lo_p = np.zeros((B, PADF), bf16)
    lo_p[:, : xf.shape[1]] = x_lo
    xr = np.empty((B, 81, HP * HP), bf16)
    r = 0
    for t, src in ((0, hi_p), (1, hi_p), (2, lo_p)):
        for c in range(C_IN):
            for dy in range(3):
                for dx in range(3):
                    base = c * HP * HP + dy * HP + dx
                    xr[:, r] = src[:, base : base + HP * HP]
                    r += 1
    assert r == 81

    # conv1 fused weights: [81, 192] = 3 K-terms x (64 gate | 128 expert) cols
    wg = gw.transpose(1, 2, 3, 0).reshape(27, 64)            # (c,dy,dx) x o
    we = ew1.transpose(2, 3, 4, 0, 1).reshape(27, 128)       # (c,dy,dx) x (e,o)
    w_full = np.concatenate([wg, we], axis=1).astype(np.float32)
    w_hi, w_lo = _split_hi_lo(w_full)
    w1 = np.concatenate([w_hi, w_lo, w_hi], axis=0)          # [81, 192]

    bg = gb.reshape(64, 1).astype(np.float32)
    b1e = eb1.reshape(128, 1).astype(np.float32)             # (e,o) flat

    # conv2 kn2row weights: [e*32+i, k9, o=64] (expert e on partitions 32e..)
    w2 = ew2.transpose(0, 2, 3, 4, 1).reshape(128, 9, 64).astype(bf16)
    b2 = np.ascontiguousarray(eb2.transpose(1, 0).astype(np.float32))  # [64, 4]

    # expert FC weights, parity-packed to match h2P layout:
    # h2P[c + 64*par, e, s, j] with j = hp2*8 + wep2, wp2 = 2*wep2 + par
    ef = efw.reshape(4, 128, 64, 16, 8, 2)                   # e,o,c,hp2,wep2,par
    efwp = np.empty((4, 128, 128, 128), bf16)
    eft = ef.transpose(0, 5, 2, 3, 4, 1)                     # e,par,c,hp2,wep2,o
    efwp_v = efwp.reshape(4, 2, 64, 128, 128)                # e,par,c,j,o
    efwp_v[:] = eft.reshape(4, 2, 64, 128, 128)

    # gate FC weights, parity-packed to match gpoolP layout:
    # gpoolP[c + 64*par, s, j] with j = hp*16 + wep, wp = 2*wep + par
    gf = gfw.reshape(4, 64, 32, 16, 2)                       # e,c,hp,wep,par
    gfwp = np.empty((128, 512, 4), np.float32)
    gfwp_v = gfwp.reshape(2, 64, 512, 4)                     # par,c,j,e
    gfwp_v[:] = gf.transpose(4, 1, 2, 3, 0).reshape(2, 64, 512, 4)

    fwt = fw.T.astype(np.float32).copy()                     # [128, 10]
    fbv = fb.reshape(10, 1).astype(np.float32)

    shared = {
        "w1": np.ascontiguousarray(w1.astype(bf16)),
        "bg": bg, "b1e": b1e,
        "w2": np.ascontiguousarray(w2), "b2": np.ascontiguousarray(b2),
        "efwp": efwp, "gfwp": gfwp, "fwt": fwt, "fb": fbv,
        "efb": np.ascontiguousarray(efb.reshape(1, 4, 128).astype(bf16)),
        "gfb": np.ascontiguousarray(gfb.reshape(1, 4).astype(np.float32)),
    }
    return xr, shared




# --------------------------------------------------------------------------
# Persistent runner: trace/compile the NEFF-wrapped jax function once, then
# reuse it (repeat kernel() calls skip XLA retracing; test.py can time with
# device-resident inputs).
# --------------------------------------------------------------------------
class _Runner:
    def __init__(self, nc, n_cores):
        import jax
        from jax.experimental.shard_map import shard_map
        from jax.sharding import Mesh, PartitionSpec, NamedSharding
        from concourse import bass2jax

        bass2jax.install_neuronx_cc_hook()
        self.jax = jax
        self.nc = nc
        self.n_cores = n_cores
        partition_name = (
            nc.partition_id_tensor.name if nc.partition_id_tensor else None
        )
        in_names, out_names, out_avals, zero_outs = [], [], [], []
        for alloc in nc.m.functions[0].allocations:
            if not isinstance(alloc, mybir.MemoryLocationSet):
                continue
            name = alloc.memorylocations[0].name
            if alloc.kind == "ExternalInput":
                if name == partition_name:
                    continue
                in_names.append(name)
            elif alloc.kind == "ExternalOutput":
                out_names.append(name)
                shape = tuple(alloc.tensor_shape)
                dtype = mybir.dt.np(alloc.dtype)
                out_avals.append(jax.core.ShapedArray(shape, dtype))
                zero_outs.append(np.zeros(shape, dtype))
        self.in_names = list(in_names)
        self.out_names = out_names
        self.zero_outs = zero_outs
        n_params = len(in_names)
        all_names = in_names + out_names
        if partition_name is not None:
            all_names = all_names + [partition_name]
        donate = tuple(range(n_params, n_params + len(out_names)))
        out_avals_t = tuple(out_avals)

        def _body(*args):
            operands = list(args)
            if partition_name is not None:
                operands.append(bass2jax.partition_id_tensor())
            outs = bass2jax._bass_exec_p.bind(
                *operands,
                out_avals=out_avals_t,
                in_names=tuple(all_names),
                out_names=tuple(out_names),
                lowering_input_output_aliases=(),
                sim_require_finite=True,
                sim_require_nnan=True,
                nc=nc,
            )
            return tuple(outs)

        devices = jax.devices()[:n_cores]
        self.mesh = Mesh(np.asarray(devices), ("core",))
        self.sharding = NamedSharding(self.mesh, PartitionSpec("core"))
        in_specs = (PartitionSpec("core"),) * (n_params + len(out_names))
        out_specs = (PartitionSpec("core"),) * len(out_names)
        self.fn = jax.jit(
            shard_map(_body, mesh=self.mesh, in_specs=in_specs,
                      out_specs=out_specs, check_rep=False),
            donate_argnums=donate, keep_unused=True,
        )

    def concat_inputs(self, in_maps):
        return [
            np.concatenate([np.asarray(m[name]) for m in in_maps], axis=0)
            for name in self.in_names
        ]

    def put(self, concat_in):
        return [self.jax.device_put(a, self.sharding) for a in concat_in]

    def call(self, device_in):
        zeros = [np.zeros((self.n_cores * z.shape[0], *z.shape[1:]), z.dtype)
                 for z in self.zero_outs]
        outs = self.fn(*device_in, *zeros)
        return outs

    def run(self, in_maps):
        outs = self.call(self.put(self.concat_inputs(in_maps)))
        n = self.n_cores
        res = []
        for c in range(n):
            d = {}
            for i, name in enumerate(self.out_names):
                full = np.asarray(outs[i])
                d[name] = full.reshape(n, full.shape[0] // n, *full.shape[1:])[c]
            res.append(d)
        return res


_RUNNER = None


def get_runner():
    global _RUNNER
    if _RUNNER is None:
        _RUNNER = _Runner(build_nc(S), N_CORES)
    return _RUNNER


def kernel(**inputs):
    xr, shared = prep_inputs(**{k: np.asarray(v) for k, v in inputs.items()})
    runner = get_runner()
    in_maps = []
    for c in range(N_CORES):
        m = dict(shared)
        m["xr"] = np.ascontiguousarray(xr[c * S : (c + 1) * S])
        in_maps.append(m)
    res = runner.run(in_maps)
    out = np.empty((B, 10), np.float32)
    for c in range(N_CORES):
        out[c * S : (c + 1) * S] = res[c]["out"].T
    return out
